# revision 1
# baseline (speedup 1.0000x reference)
"""Trainium2 Bass kernel for DynamicTaskMemoryInduction (capsule dynamic routing).

Math (reference semantics):
  Ws = W[0,:,0]  (W is a broadcast of shared weights over the in_caps axis C)
  hat_m[c,(n,d)] = m[c,:] @ Ws[(n,d),:]^T + b[0,n,c,d]      -> tm [C=64, N*D=768]
  hat_q[q,(n,d)] = q[q,:] @ Ws[(n,d),:]^T                   -> tq [Q, 768]  (c-independent)
  p = tanh(-pearson_d(tm, tq));  2x routing loop + final squash.

Because tq (and its routing updates) are c-independent, the per-(q,n,c,d)
tensors of the reference collapse to [Q,(n,d)] / [Q,(n,c)] shapes.

Key identities used on device (exact algebra, no approximation):
  - pearson numerator: num[q,n,c] = sum_d tm[n,c,d] * u[q,n,d] where
      u = tq - mean_d(tq) (centered), because sum_d u = 0.
  - recursive numerator: with u_i = lam_i * (tq_i - mean_d tq_i) (lam_i = 2^i),
      num'_{i+1} = num'_i + lam_i*(agree_i - mean_v_i * sm1)      (sm1 = sum_d tm)
      r_{i+1} = num' / sqrt(ssm * ssq(u) + lam^2 * EPS)
  - agree via the (constant) Gram matrix of tm:
      agree[q,n,c] = scale_v[q,n] * sum_{c'} coeff[q,n,c'] * G[n,c',c],
      G[n] = tm_n @ tm_n^T,  since v = scale_v * hat_v and hat_v = coeff @ tm_n.

All matmuls/transposes run as split-bf16 (x = hi + lo, both bf16; products
keep hi@hi + hi@lo + lo@hi, dropping lo@lo ~ 2^-18 relative): fp32/fp32r PE
matmuls execute ~100x slower than bf16 on this hardware path.

Sharding: data-parallel over Q across 8 cores (64 queries/core, q on SBUF
partitions). Ws/m/b replicated; hat_m recomputed on every core (it rides along
in the same matmul as hat_q: lhsT = [qT | mT] is exactly 128 columns).
"""

import numpy as np

EPS = 1e-8
Q, I, C, N, D = 512, 768, 64, 4, 192
ND, NC = N * D, N * C
NCORES = 8
QL = Q // NCORES  # 64 queries per core


def build(reps=1, stop_at="full"):
    import concourse.bacc as bacc
    import concourse.tile as tile
    import concourse.mybir as mybir
    import concourse.masks as masks

    F32 = mybir.dt.float32
    BF16 = mybir.dt.bfloat16
    AF = mybir.ActivationFunctionType
    OP = mybir.AluOpType
    AX = mybir.AxisListType

    nc = bacc.Bacc("TRN2", target_bir_lowering=False, debug=False,
                   num_devices=NCORES)

    wsh_d = nc.dram_tensor("ws_hi", [I, ND], BF16, kind="ExternalInput").ap()
    wsl_d = nc.dram_tensor("ws_lo", [I, ND], BF16, kind="ExternalInput").ap()
    qmh_d = nc.dram_tensor("qm_hi", [I, 128], BF16, kind="ExternalInput").ap()
    qml_d = nc.dram_tensor("qm_lo", [I, 128], BF16, kind="ExternalInput").ap()
    b_d = nc.dram_tensor("b_r", [C, ND], F32, kind="ExternalInput").ap()
    out_d = nc.dram_tensor("out", [QL, ND], F32, kind="ExternalOutput").ap()
    ssm_dr = nc.dram_tensor("ssm_dr", [1, NC], F32).ap()
    s1m_dr = nc.dram_tensor("s1m_dr", [1, NC], F32).ap()

    KC = I // 128  # 6 contraction chunks

    with tile.TileContext(nc) as tc:
        with tc.tile_pool(name="const", bufs=1) as cp, \
             tc.tile_pool(name="sb768", bufs=3) as sp768, \
             tc.tile_pool(name="sb256", bufs=3) as sp256, \
             tc.tile_pool(name="sbsm", bufs=3) as spsm, \
             tc.tile_pool(name="sbt", bufs=1) as spt:

            identb = cp.tile([128, 128], BF16, tag="identb")
            masks.make_identity(nc, identb[:])
            ones_col = cp.tile([128, 1], BF16, tag="ones_col")
            nc.gpsimd.memset(ones_col[:], 1.0)
            eps_t = {}
            for lam2 in (1.0, 4.0, 16.0):
                t = cp.tile([QL, 1], F32, tag=f"eps{lam2}")
                nc.gpsimd.memset(t[:], lam2 * EPS)
                eps_t[lam2] = t

            def split(x_ap, pool, tagbase, shape, eng=None):
                """x (f32 AP) -> (hi, lo) bf16 tiles."""
                e = eng or nc.vector
                hi = pool.tile(shape, BF16, tag=f"{tagbase}h")
                e.tensor_copy(hi[:], x_ap)
                lo = pool.tile(shape, BF16, tag=f"{tagbase}l")
                e.tensor_sub(lo[:], x_ap, hi[:])
                return hi, lo

            for rep in range(reps):
                # ---------- load inputs ----------
                wsh, wsl, qmh, qml = [], [], [], []
                for k in range(KC):
                    sl = slice(k * 128, (k + 1) * 128)
                    for dsrc, lst, tg, eng in (
                            (wsh_d, wsh, "wsh", nc.sync), (wsl_d, wsl, "wsl", nc.gpsimd),
                            (qmh_d, qmh, "qmh", nc.sync), (qml_d, qml, "qml", nc.gpsimd)):
                        w_k = cp.tile([128, dsrc.shape[1]], BF16, tag=f"{tg}{k}")
                        eng.dma_start(w_k[:], dsrc[sl, :])
                        lst.append(w_k)
                b_sb = cp.tile([C, ND], F32, tag="b")
                nc.gpsimd.dma_start(b_sb[:], b_d[:])

                # ---------- phase A: [hat_q; hat_m] = qmT.T @ wsT (split) ----
                with tc.tile_pool(name="psA", bufs=1, space="PSUM") as psA:
                    ps_a = psA.tile([128, ND], F32, tag="a")
                    terms = [(qmh, wsh), (qmh, wsl), (qml, wsh)]
                    nmm = KC * len(terms)
                    i_mm = 0
                    for k in range(KC):
                        for lh, rh in terms:
                            for c0, c1 in ((0, 512), (512, 768)):
                                nc.tensor.matmul(ps_a[:, c0:c1], lh[k][:],
                                                 rh[k][:, c0:c1],
                                                 start=(i_mm == 0),
                                                 stop=(i_mm == nmm - 1))
                            i_mm += 1

                    # tm = hat_m + b ; u0 = centered hat_q
                    tm = cp.tile([C, ND], F32, tag="tm")
                    nc.vector.tensor_add(tm[:], ps_a[64:128, :], b_sb[:])

                    s1q = spsm.tile([QL, N], F32, tag="s1q")
                    nc.vector.tensor_reduce(
                        out=s1q[:], in_=ps_a[0:64, :].rearrange("p (n d) -> p n d", n=N),
                        axis=AX.X, op=OP.add)
                    muq = spsm.tile([QL, N], F32, tag="muq")
                    nc.vector.tensor_scalar_mul(muq[:], s1q[:], 1.0 / D)
                    u = sp768.tile([QL, ND], F32, tag="u")
                    nc.vector.tensor_sub(
                        u[:].rearrange("p (n d) -> p n d", n=N),
                        ps_a[0:64, :].rearrange("p (n d) -> p n d", n=N),
                        muq[:].unsqueeze(2).broadcast_to([QL, N, D]))

                if stop_at == "phaseA":
                    nc.sync.dma_start(out_d[:], tm[:])
                    continue

                # ---------- tm statistics (first: ssm_b gates p0) ----------
                sq2m = spsm.tile([C, N], F32, tag="sq2m")
                for n in range(N):
                    sl = tm[:, n * D:(n + 1) * D]
                    junkm = sp768.tile([QL, D], F32, tag=f"junk_{n}")
                    nc.scalar.activation(junkm[:], sl, AF.Square,
                                         accum_out=sq2m[:, n:n + 1])
                s1m = spsm.tile([C, N], F32, tag="s1m")
                nc.vector.tensor_reduce(
                    out=s1m[:], in_=tm[:].rearrange("p (n d) -> p n d", n=N),
                    axis=AX.X, op=OP.add)
                # ssm = sum tm^2 - (sum tm)^2 / D
                s1m2 = spsm.tile([C, N], F32, tag="s1m2")
                nc.vector.tensor_mul(s1m2[:], s1m[:], s1m[:])
                ssm = spsm.tile([C, N], F32, tag="ssm")
                nc.vector.scalar_tensor_tensor(
                    out=ssm[:], in0=s1m2[:], scalar=-1.0 / D, in1=sq2m[:],
                    op0=OP.mult, op1=OP.add)
                # ssm [64(c),4(n)] -> DRAM bounce (strided scatter write,
                # contiguous read) -> [1,(n,c)] -> partition_broadcast.
                ssm_b = cp.tile([QL, NC], F32, tag="ssm_b")
                nc.sync.dma_start(
                    out=ssm_dr[:].rearrange("x (n c) -> x c n", n=N), in_=ssm[:])
                row = spsm.tile([1, NC], F32, tag="row")
                nc.sync.dma_start(out=row[:], in_=ssm_dr[:])
                nc.gpsimd.partition_broadcast(ssm_b[:], row[:])

                tm_h, tm_l = split(tm[:], cp, "tms", [C, ND], eng=nc.gpsimd)
                u_h, u_l = split(u[:], spt, "us", [QL, ND], eng=nc.gpsimd)

                # ssq0 = sum_d u^2 per n
                ssq = spsm.tile([QL, N], F32, tag="ssq")
                for n in range(N):
                    sl = u[:, n * D:(n + 1) * D]
                    junk0 = sp768.tile([QL, D], F32, tag=f"junk_{n}")
                    nc.scalar.activation(junk0[:], sl, AF.Square,
                                         accum_out=ssq[:, n:n + 1])

                with tc.tile_pool(name="psT", bufs=2, space="PSUM") as psT, \
                     tc.tile_pool(name="psB", bufs=2, space="PSUM") as psB:
                    # transposed tiles per d-chunk: A = d 0:128, B = d 128:192,
                    # for hi and lo; columns packed [d, (n,*)] with n at cols n*64.
                    def tr_blocks(hi, lo, pool, pfx):
                        res = {}
                        for cname, off, w in (("A", 0, 128), ("B", 128, 64)):
                            for sname, src in (("h", hi), ("l", lo)):
                                pt = psT.tile([128, NC], BF16, tag=f"tr{cname}")
                                for n in range(N):
                                    nc.tensor.transpose(
                                        pt[:w, n * C:(n + 1) * C],
                                        src[:, n * D + off:n * D + off + w],
                                        identb[:64, :64])
                                t_b = pool.tile([w, NC], BF16,
                                                tag=f"{pfx}{cname}{sname}")
                                nc.vector.tensor_copy(t_b[:], pt[:w, :])
                                res[cname + sname] = t_b
                        return res

                    tmT = tr_blocks(tm_h, tm_l, cp, "tmT")
                    uT = tr_blocks(u_h, u_l, spt, "uT")

                    # sm1_b row: ones^T @ tmT (hi+lo, both d-chunks) -> [1,256]
                    sm1_b = cp.tile([QL, NC], F32, tag="sm1_b")
                    pr_s1 = psB.tile([1, NC], F32, tag="s1row")
                    for j, key in enumerate(("Ah", "Al", "Bh", "Bl")):
                        w = 128 if key[0] == "A" else 64
                        nc.tensor.matmul(pr_s1[:], ones_col[:w, :], tmT[key][:w, :],
                                         start=(j == 0), stop=(j == 3))
                    row_s1 = spsm.tile([1, NC], F32, tag="row_s1")
                    nc.vector.tensor_copy(row_s1[:], pr_s1[:])
                    nc.gpsimd.partition_broadcast(sm1_b[:], row_s1[:])

                    def mm3_blocks(out_ps, Lt, Rt, n):
                        """accumulate split product over d-chunks A,B for block n"""
                        sl = (slice(None), slice(n * C, (n + 1) * C))
                        combos = [("A", "h", "h"), ("A", "h", "l"), ("A", "l", "h"),
                                  ("B", "h", "h"), ("B", "h", "l"), ("B", "l", "h")]
                        for j, (cn, a, bside) in enumerate(combos):
                            w = 128 if cn == "A" else 64
                            nc.tensor.matmul(out_ps[sl],
                                             Lt[cn + a][:w, n * C:(n + 1) * C],
                                             Rt[cn + bside][:w, n * C:(n + 1) * C],
                                             start=(j == 0), stop=(j == len(combos) - 1))

                    # gram G[n] = tm_n @ tm_n^T
                    pg = psB.tile([C, NC], F32, tag="blk")
                    for n in range(N):
                        mm3_blocks(pg, tmT, tmT, n)
                    g_h, g_l = split(pg[:], cp, "gs", [C, NC])

                    # pear #1: num0[q,(n,c)] = sum_d u0T[d,q] * tmT[d,c]
                    pp = psB.tile([QL, NC], F32, tag="blk")
                    for n in range(N):
                        mm3_blocks(pp, uT, tmT, n)
                    num = sp256.tile([QL, NC], F32, tag="num")
                    nc.vector.tensor_copy(num[:], pp[:])

                if stop_at == "setup":
                    nc.sync.dma_start(out_d[:], u[:])
                    continue

                def make_p(num_t, ssq_t, lam):
                    """p = tanh(-num / sqrt(ssm*ssq + lam^2*EPS)) ; [64,256].
                    Only ln/exp transcendentals (activation table set 6) --
                    sqrt/tanh would force ~1.3us table reloads on ACT."""
                    den2 = sp256.tile([QL, NC], F32, tag="den2")
                    nc.vector.tensor_mul(
                        den2[:].rearrange("p (n c) -> p n c", n=N),
                        ssm_b[:].rearrange("p (n c) -> p n c", n=N),
                        ssq_t[:].unsqueeze(2).broadcast_to([QL, N, C]))
                    l_t = sp256.tile([QL, NC], F32, tag="den")
                    nc.scalar.activation(l_t[:], den2[:], AF.Ln,
                                         bias=eps_t[lam * lam][:], scale=1.0)
                    rsq = sp256.tile([QL, NC], F32, tag="inv")
                    nc.scalar.activation(rsq[:], l_t[:], AF.Exp, bias=0.0, scale=-0.5)
                    r_t = sp256.tile([QL, NC], F32, tag="r")
                    nc.vector.tensor_mul(r_t[:], num_t[:], rsq[:])
                    # tanh(-r) = 1 - 2/(1 + e^{-2r})
                    e2 = sp256.tile([QL, NC], F32, tag="e2")
                    nc.scalar.activation(e2[:], r_t[:], AF.Exp, bias=0.0, scale=-2.0)
                    t1 = sp256.tile([QL, NC], F32, tag="t1p")
                    nc.vector.tensor_scalar_add(t1[:], e2[:], 1.0)
                    t1r = sp256.tile([QL, NC], F32, tag="t1pr")
                    nc.vector.reciprocal(t1r[:], t1[:])
                    p_t = sp256.tile([QL, NC], F32, tag="p")
                    nc.vector.tensor_scalar(out=p_t[:], in0=t1r[:], scalar1=-2.0,
                                            scalar2=1.0, op0=OP.mult, op1=OP.add)
                    return p_t

                def softmax_n(a_t):
                    """softmax over n of a [64,(n,c)] -> d_sm [64,256].
                    No max-subtraction: |a| <= sum|p*agree| <= ~36 (||v||<1,
                    |p|<1, |agree| <= ||tm_n,c||*||v||), exp is fp32-safe."""
                    e_t = sp256.tile([QL, NC], F32, tag="e")
                    nc.scalar.activation(e_t[:], a_t[:], AF.Exp, bias=0.0, scale=1.0)
                    rs = spsm.tile([QL, C], F32, tag="rs")
                    nc.vector.tensor_reduce(
                        out=rs[:], in_=e_t[:].rearrange("p (n c) -> p c n", n=N),
                        axis=AX.X, op=OP.add)
                    rsi = spsm.tile([QL, C], F32, tag="rsi")
                    nc.vector.reciprocal(rsi[:], rs[:])
                    d_sm = sp256.tile([QL, NC], F32, tag="dsm")
                    nc.vector.tensor_mul(
                        d_sm[:].rearrange("p (n c) -> p n c", n=N),
                        e_t[:].rearrange("p (n c) -> p n c", n=N),
                        rsi[:].unsqueeze(1).broadcast_to([QL, N, C]))
                    return d_sm

                p_t = make_p(num, ssq, 1.0)
                a_t = None

                with tc.tile_pool(name="psI", bufs=2, space="PSUM") as psI, \
                     tc.tile_pool(name="psH", bufs=1, space="PSUM") as psH:

                    def coeff_T(coeff_t):
                        """split coeff + PE-transpose blocks -> cT (bf16 hi/lo)."""
                        c_h, c_l = split(coeff_t[:], sp256, "cs", [QL, NC])
                        cT = {}
                        for sname, src in (("h", c_h), ("l", c_l)):
                            pc = psI.tile([64, NC], BF16, tag="ctr")
                            for n in range(N):
                                nc.tensor.transpose(pc[:, n * C:(n + 1) * C],
                                                    src[:, n * C:(n + 1) * C],
                                                    identb[:64, :64])
                            t_c = sp256.tile([64, NC], BF16, tag=f"cT{sname}")
                            nc.vector.tensor_copy(t_c[:], pc[:])
                            cT[sname] = t_c
                        return cT

                    def hv_mm(cT):
                        hv = []
                        for n in range(N):
                            hv_n = psH.tile([QL, D], F32, tag=f"hv{n}")
                            csl = (slice(None), slice(n * C, (n + 1) * C))
                            dsl = (slice(None), slice(n * D, (n + 1) * D))
                            nc.tensor.matmul(hv_n[:], cT["h"][csl], tm_h[dsl],
                                             start=True, stop=False)
                            nc.tensor.matmul(hv_n[:], cT["h"][csl], tm_l[dsl],
                                             start=False, stop=False)
                            nc.tensor.matmul(hv_n[:], cT["l"][csl], tm_h[dsl],
                                             start=False, stop=True)
                            hv.append(hv_n)
                        return hv

                    def agree_mm(cT):
                        pag = psI.tile([QL, NC], F32, tag="ag")
                        for n in range(N):
                            csl = (slice(None), slice(n * C, (n + 1) * C))
                            nc.tensor.matmul(pag[csl], cT["h"][csl], g_h[csl],
                                             start=True, stop=False)
                            nc.tensor.matmul(pag[csl], cT["h"][csl], g_l[csl],
                                             start=False, stop=False)
                            nc.tensor.matmul(pag[csl], cT["l"][csl], g_h[csl],
                                             start=False, stop=True)
                        return pag

                    def qform(in0_t, in1_t, tag):
                        """per-n out[q,n] = sum_c in0[q,(n,c)]*in1[q,(n,c)].
                        (scalar_tensor_tensor with mult/mult == fused mul-reduce;
                        InstTensorTensorReduce faults on this hardware path.)"""
                        res = spsm.tile([QL, N], F32, tag=tag)
                        for n in range(N):
                            sl = (slice(None), slice(n * C, (n + 1) * C))
                            junkq = spsm.tile([QL, C], F32, tag=f"junkq_{n}")
                            nc.vector.scalar_tensor_tensor(
                                out=junkq[:], in0=in0_t[sl], scalar=1.0,
                                in1=in1_t[sl], op0=OP.mult, op1=OP.mult,
                                accum_out=res[:, n:n + 1])
                        return res

                    def scale_from_sshv(sshv):
                        """squash scale = (sq/(1+sq))/sqrt(sq+EPS)"""
                        t1 = spsm.tile([QL, N], F32, tag="t1")
                        nc.vector.tensor_scalar_add(t1[:], sshv[:], 1.0)
                        t1r = spsm.tile([QL, N], F32, tag="t1r")
                        nc.vector.reciprocal(t1r[:], t1[:])
                        t2 = spsm.tile([QL, N], F32, tag="t2")
                        nc.vector.tensor_mul(t2[:], sshv[:], t1r[:])
                        lsq = spsm.tile([QL, N], F32, tag="ds")
                        nc.scalar.activation(lsq[:], sshv[:], AF.Ln,
                                             bias=eps_t[1.0][:], scale=1.0)
                        dsr = spsm.tile([QL, N], F32, tag="dsr")
                        nc.scalar.activation(dsr[:], lsq[:], AF.Exp, bias=0.0, scale=-0.5)
                        scale = spsm.tile([QL, N], F32, tag="scale")
                        nc.vector.tensor_mul(scale[:], t2[:], dsr[:])
                        return scale

                    lam = 1.0
                    for it in (1, 2):
                        coeff = sp256.tile([QL, NC], F32, tag="coeff")
                        if it == 1:
                            nc.vector.tensor_scalar_add(coeff[:], p_t[:], 1.0 / N)
                        else:
                            d_sm = softmax_n(a_t)
                            nc.vector.tensor_add(coeff[:], d_sm[:], p_t[:])

                        cT = coeff_T(coeff)
                        pag = agree_mm(cT)
                        # quadratic-form stats (no hv needed):
                        #   sshv = sum_c coeff*(coeff@G), s1hv = sum_c coeff*sm1,
                        #   sum_d u*hv = sum_c coeff*num'
                        sshv = qform(coeff[:], pag[:], "sshv")
                        s1hv = qform(coeff[:], sm1_b[:], "s1hv")
                        qf1 = qform(coeff[:], num[:], "qf1")
                        scale = scale_from_sshv(sshv)

                        # agree = scale_v (bcast c) * pag
                        agree = sp256.tile([QL, NC], F32, tag="agree")
                        nc.vector.tensor_mul(
                            agree[:].rearrange("p (n c) -> p n c", n=N),
                            pag[:].rearrange("p (n c) -> p n c", n=N),
                            scale[:].unsqueeze(2).broadcast_to([QL, N, C]))

                        # a update: a += p * agree
                        pa = sp256.tile([QL, NC], F32, tag="pa")
                        nc.vector.tensor_mul(pa[:], p_t[:], agree[:])
                        if it == 1:
                            a_t = pa
                        else:
                            a_new = sp256.tile([QL, NC], F32, tag="a")
                            nc.vector.tensor_add(a_new[:], a_t[:], pa[:])
                            a_t = a_new

                        # mean_v = (s1hv/D) * scale
                        mv = spsm.tile([QL, N], F32, tag="mv")
                        nc.vector.scalar_tensor_tensor(
                            out=mv[:], in0=s1hv[:], scalar=1.0 / D, in1=scale[:],
                            op0=OP.mult, op1=OP.mult)

                        # num' += lam * (agree - mv*sm1)
                        q1 = sp256.tile([QL, NC], F32, tag="q1")
                        nc.vector.tensor_mul(
                            q1[:].rearrange("p (n c) -> p n c", n=N),
                            sm1_b[:].rearrange("p (n c) -> p n c", n=N),
                            mv[:].unsqueeze(2).broadcast_to([QL, N, C]))
                        q2 = sp256.tile([QL, NC], F32, tag="q2")
                        nc.vector.tensor_sub(q2[:], agree[:], q1[:])
                        num_new = sp256.tile([QL, NC], F32, tag="num")
                        nc.vector.scalar_tensor_tensor(
                            out=num_new[:], in0=q2[:], scalar=lam, in1=num[:],
                            op0=OP.mult, op1=OP.add)

                        # ssq' = ssq + 2*lam*T1 + lam^2*T2 with
                        #   T1 = scale*qf1 (= sum_d u*w), T2 = scale^2*sshv - D*mv^2
                        t1s = spsm.tile([QL, N], F32, tag="t1s")
                        nc.vector.tensor_mul(t1s[:], scale[:], qf1[:])
                        m1 = spsm.tile([QL, N], F32, tag="m1")
                        nc.vector.tensor_mul(m1[:], scale[:], scale[:])
                        m2 = spsm.tile([QL, N], F32, tag="m2")
                        nc.vector.tensor_mul(m2[:], m1[:], sshv[:])
                        m3 = spsm.tile([QL, N], F32, tag="m3")
                        nc.vector.tensor_mul(m3[:], mv[:], mv[:])
                        t2s = spsm.tile([QL, N], F32, tag="t2s")
                        nc.vector.scalar_tensor_tensor(
                            out=t2s[:], in0=m3[:], scalar=-float(D), in1=m2[:],
                            op0=OP.mult, op1=OP.add)
                        x1 = spsm.tile([QL, N], F32, tag="x1")
                        nc.vector.scalar_tensor_tensor(
                            out=x1[:], in0=t1s[:], scalar=2.0 * lam, in1=ssq[:],
                            op0=OP.mult, op1=OP.add)
                        ssq_new = spsm.tile([QL, N], F32, tag="ssq")
                        nc.vector.scalar_tensor_tensor(
                            out=ssq_new[:], in0=t2s[:], scalar=lam * lam, in1=x1[:],
                            op0=OP.mult, op1=OP.add)
                        ssq = ssq_new
                        num = num_new
                        lam *= 2.0
                        p_t = make_p(num, ssq, lam)

                    # ---------- final: d=softmax(a), hv3, squash -> out ----------
                    d_sm = softmax_n(a_t)
                    coeff = sp256.tile([QL, NC], F32, tag="coeff")
                    nc.vector.tensor_add(coeff[:], d_sm[:], p_t[:])
                    cT = coeff_T(coeff)
                    hv = hv_mm(cT)
                    sshv3 = spsm.tile([QL, N], F32, tag="sshv")
                    for n in range(N):
                        junk2 = sp768.tile([QL, D], F32, tag=f"junk2_{n}")
                        nc.scalar.activation(junk2[:], hv[n][:], AF.Square,
                                             accum_out=sshv3[:, n:n + 1])
                    scale = scale_from_sshv(sshv3)
                    out_sb = sp768.tile([QL, ND], F32, tag="out")
                    for n in range(N):
                        nc.vector.tensor_scalar_mul(
                            out_sb[:, n * D:(n + 1) * D],
                            hv[n][:], scale[:, n:n + 1])
                    nc.sync.dma_start(out_d[:], out_sb[:])

    # All our activation funcs (Ln/Exp/Square/Identity/Copy) live together in
    # the 'natural_log_exp_and_others' table set, but insert_act_table_loads
    # greedily assigns Ln and Exp to different sets and thrashes ~13 table
    # loads (~1.3us each). During compile, advertise funcs only for the
    # combined set (list order/indices preserved) so the pass hoists a single
    # load.
    import concourse.bacc as bacc_mod
    from concourse.hw_specs import get_activation_tables as _real_gat

    def _gat_combined_only(arch):
        tables = _real_gat(arch)
        return {name: (funcs if name == "natural_log_exp_and_others" else set())
                for name, funcs in tables.items()}

    bacc_mod.get_activation_tables = _gat_combined_only
    try:
        nc.compile()
    finally:
        bacc_mod.get_activation_tables = _real_gat
    return nc


_BUILD_CACHE = {}


def _get_built(reps=1):
    if reps not in _BUILD_CACHE:
        _BUILD_CACHE[reps] = build(reps)
    return _BUILD_CACHE[reps]


def _split_np(x):
    import ml_dtypes
    hi = x.astype(ml_dtypes.bfloat16)
    lo = (x - hi.astype(np.float32)).astype(ml_dtypes.bfloat16)
    return hi, lo


def _prep_inputs(m, q, W, b):
    """Host-side layout prep + per-core sharding."""
    m = np.asarray(m, dtype=np.float32)
    q = np.asarray(q, dtype=np.float32)
    W = np.asarray(W, dtype=np.float32)
    b = np.asarray(b, dtype=np.float32)
    Ws = W[0, :, 0, :, :].reshape(ND, I)          # [N*D, I]
    wsT = np.ascontiguousarray(Ws.T)              # [I, N*D]
    ws_hi, ws_lo = _split_np(wsT)
    mT = m.T                                      # [I, C]
    b_r = np.ascontiguousarray(b[0].transpose(1, 0, 2).reshape(C, ND))
    in_maps = []
    for c in range(NCORES):
        qc = q[c * QL:(c + 1) * QL, :]            # [QL, I]
        qmT = np.ascontiguousarray(np.concatenate([qc.T, mT], axis=1))  # [I, 128]
        qm_hi, qm_lo = _split_np(qmT)
        in_maps.append({"ws_hi": ws_hi, "ws_lo": ws_lo,
                        "qm_hi": qm_hi, "qm_lo": qm_lo, "b_r": b_r})
    return in_maps


def kernel(m, q, W, b):
    from concourse.bass_utils import run_bass_kernel_spmd
    nc = _get_built(1)
    in_maps = _prep_inputs(m, q, W, b)
    res = run_bass_kernel_spmd(nc, in_maps, list(range(NCORES)))
    out = np.concatenate([res.results[c]["out"] for c in range(NCORES)], axis=0)
    return out.astype(np.float32)

